# revision 14
# baseline (speedup 1.0000x reference)
"""Multi-head attention (B=8, T=1024, E=1024, H=16, D=64) on 8 TRN2 cores.

Strategy: data-parallel over batch (core c gets batch c), no collectives.
Per-core single-batch attention, all matmuls in float32r (1 cyc/row):
  - xT via PE transposes (is_transpose fp32 = 2 cyc/row)
  - qT/kT head-major [hd, t] via stationary w col-blocks x moving xT
  - RoPE: DVE stream_shuffle pair-swap + cos/sin tables (pre-scaled 1/sqrt(8))
  - logitsT [T_p, t_f] per (head, T-block); exp on ScalarE (no max-subtraction:
    logits are ~N(0, 0.4^2), exp is safe, softmax is shift-invariant)
  - AV with ones-augmented V stationary -> PSUM [65, 1024] accumulates out'T_h
    plus the softmax denominator row for free
  - normalize: DVE reciprocal + PE ones-outer-product broadcast + fused DVE mult
  - final proj: stationary outT blocks x moving w_o rows
"""

import os
import sys

import numpy as np

for _p in ("/opt/trn_rl_repo", "/root/.axon_site/_ro/trn_rl_repo"):
    if os.path.isdir(_p) and _p not in sys.path:
        sys.path.insert(0, _p)

NUM_HEADS = 16
HEAD_DIM = 64
MAX_TIME = 10000.0
B, T, E = 8, 1024, 1024
N_CORES = 8
P = 128  # partitions

_CACHE = {}


def _rope_tables():
    """cos/sin tables [128, 1024]: row p = d (p % 64), col t.

    Pre-scaled by 1/sqrt(8) each (so q.k picks up the 1/sqrt(64) logit scale),
    sin pre-signed: row 2j gets -sin (pairs (x0,x1)->(x0 c - x1 s, x1 c + x0 s),
    and stream_shuffle swaps pairs so shuf[2j]=x[2j+1], shuf[2j+1]=x[2j])."""
    pos = np.arange(T, dtype=np.float64)
    freq_seq = np.arange(HEAD_DIM // 2, dtype=np.float64) / (HEAD_DIM // 2)
    inv_freq = np.repeat(MAX_TIME ** (-freq_seq), 2)  # [64]
    theta = pos[None, :] * inv_freq[:, None]  # [64, 1024]
    s = 1.0 / np.sqrt(np.sqrt(64.0))  # 1/sqrt(8) per table
    cos = (np.cos(theta) * s).astype(np.float32)
    sin = (np.sin(theta) * s).astype(np.float32)
    sign = np.where(np.arange(HEAD_DIM) % 2 == 0, -1.0, 1.0).astype(np.float32)
    sin = sin * sign[:, None]
    cos2 = np.concatenate([cos, cos], axis=0)  # [128, 1024] (2 heads per tile)
    sin2 = np.concatenate([sin, sin], axis=0)
    return np.ascontiguousarray(cos2), np.ascontiguousarray(sin2)


def _build():
    import concourse.bass as bass
    import concourse.mybir as mybir
    import concourse.tile as tile
    from concourse import bacc

    f32 = mybir.dt.float32
    f32r = mybir.dt.float32r
    Exp = mybir.ActivationFunctionType.Exp

    nc = bacc.Bacc("TRN2", target_bir_lowering=False, debug=False,
                   num_devices=N_CORES)

    xq_d = nc.dram_tensor("xq", [T, E], f32, kind="ExternalInput").ap()
    xkv_d = nc.dram_tensor("xkv", [T, E], f32, kind="ExternalInput").ap()
    wq_d = nc.dram_tensor("wq", [E, E], f32r, kind="ExternalInput").ap()
    wk_d = nc.dram_tensor("wk", [E, E], f32r, kind="ExternalInput").ap()
    wv_d = nc.dram_tensor("wv", [E, E], f32r, kind="ExternalInput").ap()
    wo_d = nc.dram_tensor("wo", [E, E], f32r, kind="ExternalInput").ap()
    cos_d = nc.dram_tensor("cos", [P, T], f32, kind="ExternalInput").ap()
    sin_d = nc.dram_tensor("sin", [P, T], f32, kind="ExternalInput").ap()
    id_d = nc.dram_tensor("ident", [P, P], f32, kind="ExternalInput").ap()
    out_d = nc.dram_tensor("out", [T, E], f32, kind="ExternalOutput").ap()

    def bc(ap):
        return ap

    with tile.TileContext(nc) as tc:
        with (
            tc.tile_pool(name="consts", bufs=1) as cpool,
            tc.tile_pool(name="big", bufs=1) as big,
            tc.tile_pool(name="wcol", bufs=3) as wcol_p,
            tc.tile_pool(name="stage", bufs=3) as stage_p,
            tc.tile_pool(name="rope", bufs=3) as rope_p,
            tc.tile_pool(name="small", bufs=2) as small_p,
        ):
            cos_t = cpool.tile([P, T], f32)
            sin_t = cpool.tile([P, T], f32)
            ident2 = cpool.tile([P, P], f32)
            ones_f = cpool.tile([P, P], f32)
            ones = cpool.tile([1, HEAD_DIM], f32r)
            nc.sync.dma_start(cos_t[:], cos_d[:])
            nc.sync.dma_start(sin_t[:], sin_d[:])
            idst = stage_p.tile([P, T], f32, tag="stage", name="idst")
            nc.sync.dma_start(idst[:, 0:P], id_d[:])
            nc.vector.memset(ones_f[:], 1.0)
            nc.vector.tensor_copy(ones[:], ones_f[0:1, 0:HEAD_DIM])
            nc.vector.tensor_copy(ident2[:], idst[:, 0:P])

            # persistent big tiles
            xT = big.tile([P, 8, T], f32r, tag="X", name="xkvT")  # [e_p, e_blk, t]
            qT = big.tile([P, 8, T], f32r, name="qT")
            kT = big.tile([P, 8, T], f32r, name="kT")
            vv = big.tile([P, 8, NUM_HEADS, HEAD_DIM + 1], f32r, name="vv")
            wrow = big.tile([P, 8, T], f32r, tag="WR", name="wv_rows")

            xq_r = xq_d.rearrange("(i p) n -> i p n", p=P)
            xkv_r = xkv_d.rearrange("(i p) n -> i p n", p=P)
            wv_r = wv_d.rearrange("(i p) n -> i p n", p=P)
            wo_r = wo_d.rearrange("(i p) n -> i p n", p=P)
            out_r = out_d.rearrange("(b p) n -> b p n", p=P)

            def transpose_into(xT_tile, x_r, ps_pool):
                for i in range(8):  # t-block rows of x
                    st = stage_p.tile([P, T], f32, tag="stage", name=f"xs{i}")
                    nc.sync.dma_start(st[:], x_r[i])
                    for j in range(8):  # e blocks
                        pt = ps_pool.tile([P, P], f32, tag="tp", name=f"tp{i}{j}")
                        nc.tensor.transpose(pt[:], st[:, j * P:(j + 1) * P], ident2[:])
                        nc.vector.tensor_copy(
                            xT_tile[:, j, i * P:(i + 1) * P], pt[:])

            def qk_proj(dst, w_dram, ps_pool):
                # dst[hd_p, hd_blk j, t] = roped( sum_e w[e, hd_j] * xT[e, t] )
                w_r = w_dram.rearrange("(i p) (j c) -> j p i c", p=P, c=P)
                for j in range(8):
                    wc2 = wcol_p.tile([P, 8, P], f32r, tag="wc", name=f"wc{j}")
                    nc.sync.dma_start(wc2[:], w_r[j])
                    for s in range(2):
                        sl = slice(s * 512, (s + 1) * 512)
                        ps = ps_pool.tile([P, 512], f32, tag="proj",
                                          name=f"pj{j}{s}")
                        for i in range(8):
                            nc.tensor.matmul(ps[:], bc(wc2[:, i, :]),
                                             bc(xT[:, i, sl]),
                                             start=(i == 0), stop=(i == 7))
                        # RoPE: dst = ps*cos + shuffle(ps)*sin_signed
                        shuf = rope_p.tile([P, 512], f32, tag="rp",
                                           name=f"sh{j}{s}")
                        tmp = rope_p.tile([P, 512], f32, tag="rp",
                                          name=f"tm{j}{s}")
                        mask = []
                        for g in range(16):
                            mask += [2 * g + 1, 2 * g]
                        tmp2 = rope_p.tile([P, 512], f32, tag="rp",
                                           name=f"tc{j}{s}")
                        nc.vector.stream_shuffle(shuf[:], ps[:], mask)
                        nc.vector.tensor_mul(tmp[:], shuf[:], sin_t[:, sl])
                        nc.vector.tensor_mul(tmp2[:], ps[:], cos_t[:, sl])
                        nc.vector.tensor_add(dst[:, j, sl], tmp2[:], tmp[:])

            with tc.tile_pool(name="ps1", space="PSUM", bufs=2) as ps1:
                transpose_into(xT, xkv_r, ps1)
                qk_proj(kT, wk_d, ps1)
                # v in t-major layout with ones column per head stripe
                for i in range(8):
                    nc.sync.dma_start(wrow[:, i, :], wv_r[i])
                for b in range(8):  # T blocks
                    for s in range(2):  # hd slices
                        ps = ps1.tile([P, 512], f32, tag="proj", name=f"pv{b}{s}")
                        for i in range(8):
                            nc.tensor.matmul(
                                ps[:], bc(xT[:, i, b * P:(b + 1) * P]),
                                bc(wrow[:, i, s * 512:(s + 1) * 512]),
                                start=(i == 0), stop=(i == 7))
                        nc.vector.tensor_copy(
                            vv[:, b, 8 * s:8 * s + 8, 0:HEAD_DIM],
                            ps.rearrange("p (h c) -> p h c", c=HEAD_DIM))
                nc.vector.tensor_copy(
                    vv[:, :, :, HEAD_DIM],
                    ones_f[:, 0:NUM_HEADS * 8].rearrange(
                        "p (b h) -> p b h", b=8))
                # now overwrite xT slot with xq's transpose, then q proj
                xT2 = big.tile([P, 8, T], f32r, tag="X", name="xqT")
                transpose_into(xT2, xq_r, ps1)
                # q proj reads xT2
                w_r = wq_d.rearrange("(i p) (j c) -> j p i c", p=P, c=P)
                for j in range(8):
                    wc2 = wcol_p.tile([P, 8, P], f32r, tag="wc", name=f"wq{j}")
                    nc.sync.dma_start(wc2[:], w_r[j])
                    for s in range(2):
                        sl = slice(s * 512, (s + 1) * 512)
                        ps = ps1.tile([P, 512], f32, tag="proj", name=f"pq{j}{s}")
                        for i in range(8):
                            nc.tensor.matmul(ps[:], bc(wc2[:, i, :]),
                                             bc(xT2[:, i, sl]),
                                             start=(i == 0), stop=(i == 7))
                        shuf = rope_p.tile([P, 512], f32, tag="rp", name=f"qsh{j}{s}")
                        tmp = rope_p.tile([P, 512], f32, tag="rp", name=f"qtm{j}{s}")
                        mask = []
                        for g in range(16):
                            mask += [2 * g + 1, 2 * g]
                        tmp2 = rope_p.tile([P, 512], f32, tag="rp",
                                           name=f"qtc{j}{s}")
                        nc.vector.stream_shuffle(shuf[:], ps[:], mask)
                        nc.vector.tensor_mul(tmp[:], shuf[:], sin_t[:, sl])
                        nc.vector.tensor_mul(tmp2[:], ps[:], cos_t[:, sl])
                        nc.vector.tensor_add(qT[:, j, sl], tmp2[:], tmp[:])

            # load w_o rows into the wrow slot (waits for v-proj's last read)
            wrow2 = big.tile([P, 8, T], f32r, tag="WR", name="wo_rows")
            for i in range(8):
                nc.sync.dma_start(wrow2[:, i, :], wo_r[i])
            outT = big.tile([P, 8, T], f32r, tag="X", name="outT")

            with tc.tile_pool(name="ps2", space="PSUM", bufs=2) as ps2:
                for h in range(NUM_HEADS):
                    hb, off = h // 2, (h % 2) * HEAD_DIM
                    oacc = ps2.tile([P, T], f32, tag="oacc", name=f"oa{h}")
                    for tb in range(8):
                        lg = ps2.tile([P, T], f32, tag="lg", name=f"lg{h}{tb}")
                        for s in range(2):
                            sl = slice(s * 512, (s + 1) * 512)
                            nc.tensor.matmul(
                                lg[:, sl],
                                bc(kT[off:off + HEAD_DIM, hb, tb * P:(tb + 1) * P]),
                                bc(qT[off:off + HEAD_DIM, hb, sl]),
                                start=True, stop=True)
                        ex = stage_p.tile([P, T], f32r, tag="stage",
                                          name=f"ex{h}{tb}")
                        nc.scalar.activation(ex[:], lg[:], Exp)
                        for s in range(2):
                            sl = slice(s * 512, (s + 1) * 512)
                            nc.tensor.matmul(
                                oacc[0:HEAD_DIM + 1, sl],
                                bc(vv[:, tb, h, :]), bc(ex[:, sl]),
                                start=(tb == 0), stop=(tb == 7))
                    recip = small_p.tile([1, T], f32r, tag="rc", bufs=1, name=f"rc{h}")
                    with nc.allow_low_precision(reason="f32r recip feeds f32r bcast matmul"):
                        nc.vector.reciprocal(recip[:],
                                             oacc[HEAD_DIM:HEAD_DIM + 1, :])
                    bcast = ps2.tile([P, T], f32, tag="lg", name=f"bc{h}")
                    for s in range(2):
                        sl = slice(s * 512, (s + 1) * 512)
                        nc.tensor.matmul(bcast[0:HEAD_DIM, sl],
                                         bc(ones[:]), bc(recip[:, sl]),
                                         start=True, stop=True)
                    bcs = small_p.tile([HEAD_DIM, T], f32, tag="bcs", bufs=1,
                                       name=f"bcs{h}")
                    nc.vector.tensor_copy(bcs[:], bcast[0:HEAD_DIM, :])
                    nc.vector.tensor_mul(outT[off:off + HEAD_DIM, hb, :],
                                         oacc[0:HEAD_DIM, :], bcs[:])

                # final projection: out[t, e] = sum_hd outT[hd, t] * wo[hd, e]
                for b in range(8):
                    fp = ps2.tile([P, T], f32, tag="lg", name=f"fp{b}")
                    for s in range(2):
                        sl = slice(s * 512, (s + 1) * 512)
                        for i in range(8):
                            nc.tensor.matmul(
                                fp[:, sl], bc(outT[:, i, b * P:(b + 1) * P]),
                                bc(wrow2[:, i, sl]),
                                start=(i == 0), stop=(i == 7))
                    so = stage_p.tile([P, T], f32, tag="stage", name=f"so{b}")
                    nc.vector.tensor_copy(so[:], fp[:])
                    nc.sync.dma_start(out_r[b], so[:])

    nc.compile()
    return nc


def _get_nc():
    if "nc" not in _CACHE:
        _CACHE["nc"] = _build()
    return _CACHE["nc"]


def kernel(inputs_q, inputs_kv, w_q, w_k, w_v, w_o, trace=False, **kw):
    from concourse.bass_utils import run_bass_kernel_spmd

    nc = _get_nc()
    cos_t, sin_t = _rope_tables()
    ident = np.eye(P, dtype=np.float32)
    com = {
        "wq": np.ascontiguousarray(w_q, np.float32),
        "wk": np.ascontiguousarray(w_k, np.float32),
        "wv": np.ascontiguousarray(w_v, np.float32),
        "wo": np.ascontiguousarray(w_o, np.float32),
        "cos": cos_t, "sin": sin_t, "ident": ident,
    }
    in_maps = []
    for c in range(N_CORES):
        m = dict(com)
        m["xq"] = np.ascontiguousarray(inputs_q[c], np.float32)
        m["xkv"] = np.ascontiguousarray(inputs_kv[c], np.float32)
        in_maps.append(m)
    res = run_bass_kernel_spmd(nc, in_maps, core_ids=list(range(N_CORES)),
                               trace=trace, **kw)
    out = np.stack([res.results[c]["out"] for c in range(N_CORES)], axis=0)
    _CACHE["last_result"] = res
    return out
